# revision 4
# baseline (speedup 1.0000x reference)
"""Trainium2 Bass kernel: LSTM Encoder (keras LSTMCell, all-tanh gates).

reference computes, for x:[64,512,256], W:[256,2048], U:[512,2048], b:[2048]:
    xp = x @ W + b
    scan over t: z = xp_t + h @ U; i,f,g,o = tanh(split(z,4))
                 c = f*c + i*g; h = o*tanh(c)
    returns (concat([x_last, h_last]), concat([x, hs], -1))

Strategy: data-parallel over batch, 8 cores x 8 batch rows.
Per core, everything is computed in a transposed layout (4H/H on
partitions, batch on the free dim) so the recurrence needs no on-chip
transposes:
  - phase 1: xp^T tiles via PE (x transposed once with PE-transpose),
    stored fp16 in SBUF [128, 16*4096].
  - phase 2 scan: z^T tile [128, (m=16, b=8)] accumulated in PSUM from
    64 fp16 matmuls (U stationary -> fast-weight-load), gates tanh on
    ScalarE, state update on VectorE, c kept fp32, h written fp16 (next
    step's moving operand) + fp32 (staged -> DMA to DRAM).
Host reassembles full outputs (concat with the passthrough x).
"""

import numpy as np

import concourse.bass as bass
import concourse.mybir as mybir
import concourse.tile as tile
from concourse import bacc
from concourse import bass_utils
from concourse.masks import make_identity

F32 = mybir.dt.float32
F16 = mybir.dt.float16
TANH = mybir.ActivationFunctionType.Tanh

B, T, D, H = 64, 512, 256, 512
NCORES = 8
BL = B // NCORES          # 8 batch rows per core
H4 = 4 * H                # 2048
KC = H // 128             # 4 contraction chunks for U
DC = D // 128             # 2 contraction chunks for W
MC = H4 // 128            # 16 partition tiles of z^T
DMA_CHUNK = 8             # scan steps per hs DMA


def emit_lstm(tc, hs, x, W, U, b, t_len=T, b_is_zero=True):
    """Emit the per-core program.

    hs: [128, t_len, KC, BL] f32 out (hs[p, t, hc, bl] = h_t[bl, 128*hc+p])
    x:  [BL, t_len, D] f32; W: [D, H4] f32; U: [H, H4] f32; b: [H4] f32
    """
    nc = tc.nc
    rows = BL * t_len
    x_flat = x.rearrange("b t d -> (b t) d")

    with (
        tc.tile_pool(name="persist", bufs=1) as persist,
        tc.tile_pool(name="const", bufs=1) as const_pool,
    ):
        ident = const_pool.tile([128, 128], F16)
        make_identity(nc, ident[:])

        U_sb = persist.tile([128, KC * H4], F16)
        W_sb = persist.tile([128, DC * H4], F16)
        b_sb = persist.tile([128, MC], F32)
        xp_sb = persist.tile([128, MC, BL, t_len], F16)
        xT_sb = persist.tile([128, DC * rows], F16)

        # --- load + cast weights ---
        with tc.tile_pool(name="wload", bufs=2) as wload:
            for k in range(KC):
                stg = wload.tile([128, H4], F32, tag="wstg")
                nc.sync.dma_start(stg[:], U[128 * k : 128 * (k + 1), :])
                nc.vector.tensor_copy(U_sb[:, H4 * k : H4 * (k + 1)], stg[:])
            for k in range(DC):
                stg = wload.tile([128, H4], F32, tag="wstg")
                nc.sync.dma_start(stg[:], W[128 * k : 128 * (k + 1), :])
                nc.vector.tensor_copy(W_sb[:, H4 * k : H4 * (k + 1)], stg[:])
        nc.sync.dma_start(b_sb[:], b.rearrange("(m p) -> p m", p=128))

        # --- load x, cast fp16, transpose to xT (d on partitions) ---
        with (
            tc.tile_pool(name="xload", bufs=3) as xload,
            tc.tile_pool(name="xtp", bufs=4, space="PSUM") as xtp,
        ):
            r0 = 0
            ri = 0
            while r0 < rows:
                pr = min(128, rows - r0)
                xstg = xload.tile([128, D], F32, tag="xstg")
                nc.sync.dma_start(xstg[:pr, :], x_flat[r0 : r0 + pr, :])
                x16 = xload.tile([128, D], F16, tag="x16")
                nc.vector.tensor_copy(x16[:pr, :], xstg[:pr, :])
                for k in range(DC):
                    ps = xtp.tile([128, 128], F16, tag="tp")
                    nc.tensor.transpose(
                        ps[:, :pr], x16[:pr, 128 * k : 128 * (k + 1)], ident[:pr, :pr]
                    )
                    dst = xT_sb[:, rows * k + r0 : rows * k + r0 + pr]
                    if (ri + k) % 2 == 0:
                        nc.vector.tensor_copy(dst, ps[:, :pr])
                    else:
                        nc.scalar.copy(dst, ps[:, :pr])
                r0 += 128
                ri += 1

        # --- phase 1 matmuls: xp^T[m] = sum_k W[k,m]^T @ xT[k]  (+ b) ---
        xp_flat = xp_sb.rearrange("p m b t -> p (m b t)")
        with tc.tile_pool(name="p1psum", bufs=8, space="PSUM") as p1p:
            n_j = (rows + 511) // 512
            for mh in range(MC):
                for j in range(n_j):
                    c0 = 512 * j
                    cn = min(512, rows - c0)
                    ps = p1p.tile([128, 512], F32, tag="p1")
                    for k in range(DC):
                        nc.tensor.matmul(
                            ps[:, :cn],
                            W_sb[:, H4 * k + 128 * mh : H4 * k + 128 * (mh + 1)],
                            xT_sb[:, rows * k + c0 : rows * k + c0 + cn],
                            start=(k == 0),
                            stop=(k == DC - 1),
                        )
                    dst = xp_flat[:, rows * mh + c0 : rows * mh + c0 + cn]
                    if b_is_zero:
                        if (mh * n_j + j) % 2 == 0:
                            nc.vector.tensor_copy(dst, ps[:, :cn])
                        else:
                            nc.scalar.copy(dst, ps[:, :cn])
                    else:
                        nc.vector.tensor_scalar_add(dst, ps[:, :cn], b_sb[:, mh : mh + 1])

        # --- phase 2: the scan ---
        with (
            tc.tile_pool(name="state", bufs=2) as state_pool,
            tc.tile_pool(name="zwork", bufs=3) as zwork,
            tc.tile_pool(name="gwork", bufs=2) as gwork,
            tc.tile_pool(name="hstg", bufs=2) as hstg_pool,
            tc.tile_pool(name="zpsum", bufs=2, space="PSUM") as zpsum,
        ):
            h_prev = state_pool.tile([128, KC, BL], F16, tag="h")
            c_prev = state_pool.tile([128, KC, BL], F32, tag="c")
            nc.vector.memzero(h_prev[:])
            nc.vector.memzero(c_prev[:])

            hs_stage = None
            for t in range(t_len):
                ps = zpsum.tile([128, MC, BL], F32, tag="z")
                # m-outer: PSUM zero regions (2KB = bank) allow only one
                # open accumulation group at a time
                for m in range(MC):
                    for k in range(KC):
                        nc.tensor.matmul(
                            ps[:, m, :],
                            U_sb[:, H4 * k + 128 * m : H4 * k + 128 * (m + 1)],
                            h_prev[:, k, :],
                            start=(k == 0),
                            stop=(k == KC - 1),
                        )
                z_s = zwork.tile([128, MC, BL], F32, tag="zs")
                nc.vector.tensor_add(z_s[:], ps[:], xp_sb[:, :, :, t])
                tz = zwork.tile([128, MC, BL], F32, tag="tz")
                nc.scalar.activation(tz[:], z_s[:], TANH)
                i_s = tz[:, 0:4, :]
                f_s = tz[:, 4:8, :]
                g_s = tz[:, 8:12, :]
                o_s = tz[:, 12:16, :]
                t1 = gwork.tile([128, KC, BL], F32, tag="t1")
                t2 = gwork.tile([128, KC, BL], F32, tag="t2")
                nc.vector.tensor_mul(t1[:], f_s, c_prev[:])
                nc.vector.tensor_mul(t2[:], i_s, g_s)
                c_next = state_pool.tile([128, KC, BL], F32, tag="c")
                nc.vector.tensor_add(c_next[:], t1[:], t2[:])
                tcn = gwork.tile([128, KC, BL], F32, tag="tc")
                nc.scalar.activation(tcn[:], c_next[:], TANH)
                h_next = state_pool.tile([128, KC, BL], F16, tag="h")
                # fp16 h first (critical path), chunk-by-chunk
                for k in range(KC):
                    nc.vector.tensor_mul(h_next[:, k, :], o_s[:, k, :], tcn[:, k, :])
                # fp32 h for output
                if t % DMA_CHUNK == 0:
                    hs_stage = hstg_pool.tile([128, DMA_CHUNK, KC, BL], F32, tag="hs")
                nc.vector.tensor_mul(hs_stage[:, t % DMA_CHUNK], o_s, tcn[:])
                if t % DMA_CHUNK == DMA_CHUNK - 1:
                    t0 = t - (DMA_CHUNK - 1)
                    nc.sync.dma_start(hs[:, t0 : t0 + DMA_CHUNK], hs_stage[:])
                h_prev, c_prev = h_next, c_next


_PROGRAM_CACHE: dict = {}


def build_program(t_len=T, b_is_zero=True):
    key = (t_len, b_is_zero)
    if key in _PROGRAM_CACHE:
        return _PROGRAM_CACHE[key]
    nc = bacc.Bacc(
        "TRN2",
        target_bir_lowering=False,
        debug=False,
        enable_asserts=False,
        num_devices=NCORES,
    )
    x = nc.dram_tensor("x", [BL, t_len, D], F32, kind="ExternalInput").ap()
    W = nc.dram_tensor("W", [D, H4], F32, kind="ExternalInput").ap()
    U = nc.dram_tensor("U", [H, H4], F32, kind="ExternalInput").ap()
    b = nc.dram_tensor("b", [H4], F32, kind="ExternalInput").ap()
    hs = nc.dram_tensor("hs", [128, t_len, KC, BL], F32, kind="ExternalOutput").ap()
    with tile.TileContext(nc) as tc:
        emit_lstm(tc, hs, x, W, U, b, t_len=t_len, b_is_zero=b_is_zero)
    nc.compile()
    _PROGRAM_CACHE[key] = nc
    return nc


def _unpack_hs(hs_core, t_len):
    # hs_core: [128, t_len, KC, BL] -> [BL, t_len, H]
    return np.ascontiguousarray(hs_core.transpose(3, 1, 2, 0)).reshape(BL, t_len, H)


def run_cores(x, W, U, b, t_len=T):
    """Run the 8-core SPMD program; returns full hs [B, t_len, H] f32."""
    b_is_zero = not np.any(b)
    nc = build_program(t_len=t_len, b_is_zero=b_is_zero)
    in_maps = [
        {
            "x": np.ascontiguousarray(x[i * BL : (i + 1) * BL]),
            "W": W,
            "U": U,
            "b": b,
        }
        for i in range(NCORES)
    ]
    res = bass_utils.run_bass_kernel_spmd(nc, in_maps, core_ids=list(range(NCORES)))
    return np.concatenate(
        [_unpack_hs(r["hs"], t_len) for r in res.results], axis=0
    )


def kernel(x, W, U, b, batch=None, **_):
    x = np.ascontiguousarray(np.asarray(x, dtype=np.float32))
    W = np.ascontiguousarray(np.asarray(W, dtype=np.float32))
    U = np.ascontiguousarray(np.asarray(U, dtype=np.float32))
    b = np.ascontiguousarray(np.asarray(b, dtype=np.float32))
    hs_full = run_cores(x, W, U, b, t_len=x.shape[1])
    traj_h = np.concatenate([x, hs_full], axis=-1)
    context_state = np.concatenate([x[:, -1, :], hs_full[:, -1, :]], axis=-1)
    return context_state, traj_h
